# revision 12
# baseline (speedup 1.0000x reference)
"""DeepSpeed-style fused residual+LayerNorm+MLP block on 8 trn2 NeuronCores.

Strategy: data-parallel over tokens (B*S = 16384 -> 2048 tokens/core).
Each core runs the full fused chain with replicated weights; no collectives.

Per-core device kernel (Bass/Tile), pipelined over 4 supertiles of 512 tokens:
  A1: h = x + r + bias; LayerNorm stats (bn_stats/bn_aggr); ln -> bf16
  A2: PE-transpose ln to feature-major lnT [H, tok]; psum->sbuf copies on DVE.
      For supertile s+1 the transposes are interleaved one-per-i-chunk into
      supertile s's GEMM2 stream so the PE HAM clock stays warm.
  B:  interT[I,tok] = W1^T @ lnT (bf16 matmuls, fp32 PSUM);
      exact-erf GELU + per-I bias fused on ScalarE -> bf16
  C:  out[tok,H] = interT^T @ W2 (W2 streamed from HBM);
      epilogue adds h (+output_b folded in) and DMAs out.

DMA queue assignment (avoids HWDGE FIFO head-of-line blocking):
  sync   : x/r token loads + streamed W2 chunks (recurring input flows)
  gpsimd : identity + broadcast consts + W1 (one-time) + out stores
           (stores wait on late semaphores; nothing queued behind them)

Host-side prep (cheap, numpy): fold attn_nw into W1 rows, fold
attn_nb@W1+inter_b into a single GEMM1 bias, cast weights to bf16,
provide the 128x128 bf16 identity for PE transposes.
"""

import numpy as np
import ml_dtypes

import concourse.bass as bass
import concourse.bacc as bacc
import concourse.mybir as mybir
import concourse.tile as tile
from concourse.bass_utils import run_bass_kernel_spmd

N_CORES = 8
B, S, H, I = 4, 4096, 1024, 4096
TOK = B * S              # 16384 tokens total
TPC = TOK // N_CORES     # 2048 tokens per core
P = 128
T_TILES = TPC // P       # 16 token tiles per core
ST = 4                   # token tiles per supertile
N_SUPER = T_TILES // ST  # 4 supertiles
ST_TOK = ST * P          # 512 tokens per supertile
KO1 = H // P             # 8 contraction subtiles for GEMM1
IC = I // P              # 32 I-chunks
IG = 8                   # W1 i-groups (independent SBUF tiles for early start)
ICG = IC // IG           # 4 I-chunks per group
HCW = 512                # output column chunk (1 PSUM bank of f32)
HC = H // HCW            # 2
EPS = 1e-5

_F32 = mybir.dt.float32
_BF16 = mybir.dt.bfloat16

TRACE = False
LAST_RESULT = None


def _build_nc():
    nc = bacc.Bacc()
    x = nc.dram_tensor("x", (TPC, H), _F32, kind="ExternalInput")
    r = nc.dram_tensor("r", (TPC, H), _F32, kind="ExternalInput")
    w1 = nc.dram_tensor("w1", (H, I), _BF16, kind="ExternalInput")
    b1 = nc.dram_tensor("b1", (I,), _F32, kind="ExternalInput")
    w2 = nc.dram_tensor("w2", (I, H), _BF16, kind="ExternalInput")
    ab = nc.dram_tensor("ab", (H,), _F32, kind="ExternalInput")
    ob = nc.dram_tensor("ob", (H,), _F32, kind="ExternalInput")
    eye = nc.dram_tensor("eye", (P, P), _BF16, kind="ExternalInput")
    out = nc.dram_tensor("out", (TPC, H), _F32, kind="ExternalOutput")

    with tile.TileContext(nc) as tc:
        with (
            tc.tile_pool(name="consts", bufs=1) as consts,
            tc.tile_pool(name="w1p", bufs=1) as w1p,
            tc.tile_pool(name="w2s", bufs=8) as w2s,
            tc.tile_pool(name="hsup", bufs=2) as hsup,
            tc.tile_pool(name="xin", bufs=3) as xin,
            tc.tile_pool(name="rin", bufs=3) as rin,
            tc.tile_pool(name="lnp", bufs=4) as lnp,
            tc.tile_pool(name="lntp", bufs=2) as lntp,
            tc.tile_pool(name="intp", bufs=1) as intp,
            tc.tile_pool(name="resp", bufs=4) as resp,
            tc.tile_pool(name="stat", bufs=8) as stat,
            tc.tile_pool(name="ps_tr", bufs=2, space="PSUM") as ps_tr,
            tc.tile_pool(name="ps_g1", bufs=2, space="PSUM") as ps_g1,
            tc.tile_pool(name="ps_g2", bufs=4, space="PSUM") as ps_g2,
        ):
            eps_t = consts.tile([P, 1], _F32)
            nc.vector.memset(eps_t, EPS)

            ab_full = consts.tile([P, H], _F32)
            ab_ap = ab[:]
            nc.gpsimd.dma_start(
                out=ab_full,
                in_=bass.AP(tensor=ab_ap.tensor, offset=ab_ap.offset,
                            ap=[[0, P]] + list(ab_ap.ap)),
            )
            ident = consts.tile([P, P], _BF16)
            nc.gpsimd.dma_start(out=ident, in_=eye[:, :])
            ob_full = consts.tile([P, H], _F32)
            ob_ap = ob[:]
            nc.gpsimd.dma_start(
                out=ob_full,
                in_=bass.AP(tensor=ob_ap.tensor, offset=ob_ap.offset,
                            ap=[[0, P]] + list(ob_ap.ap)),
            )
            b1_st = consts.tile([P, IC], _F32)
            nc.gpsimd.dma_start(out=b1_st, in_=b1[:].rearrange("(i p) -> p i", p=P))

            w1r = w1[:, :].rearrange("(ko p) i -> p ko i", p=P)
            w2r = w2[:, :].rearrange("(io p) h -> p io h", p=P)

            h_sups = [None] * N_SUPER
            lnTs = [None] * N_SUPER
            ln_ts = [None] * N_SUPER
            w1_ig = [None] * IG

            def emit_a1(s):
                """loads + residual adds + LN stats + normalized bf16 tiles.
                Phase-ordered so the DVE never blocks on the single batched
                ACT-Sqrt round trip; x on sync queue, r on scalar queue so
                the two streams use independent DMA rings."""
                h_sup = hsup.tile([P, ST, H], _F32, name=f"h_sup{s}", tag="h_sup")
                mv = stat.tile([P, ST, 2], _F32, name=f"mv_{s}", tag="mv")
                for t in range(ST):
                    g = s * ST + t
                    x_t = xin.tile([P, H], _F32, name=f"x_{g}", tag="x_t")
                    nc.sync.dma_start(out=x_t, in_=x[g * P:(g + 1) * P, :])
                    r_t = rin.tile([P, H], _F32, name=f"r_{g}", tag="r_t")
                    nc.scalar.dma_start(out=r_t, in_=r[g * P:(g + 1) * P, :])
                    h_sl = h_sup[:, t, :]
                    nc.vector.tensor_add(h_sl, x_t, r_t)
                    nc.vector.tensor_add(h_sl, h_sl, ab_full)

                    stats = stat.tile([P, 2, 6], _F32, name=f"st_{g}", tag="stats")
                    for q in range(2):
                        nc.vector.bn_stats(out=stats[:, q, :],
                                           in_=h_sl[:, q * 512:(q + 1) * 512])
                    nc.vector.bn_aggr(out=mv[:, t, :], in_=stats)
                # one sqrt(var+eps) + reciprocal for the whole supertile
                nc.scalar.activation(out=mv[:, :, 1], in_=mv[:, :, 1],
                                     func=mybir.ActivationFunctionType.Sqrt,
                                     bias=eps_t, scale=1.0)
                nc.vector.reciprocal(out=mv[:, :, 1], in_=mv[:, :, 1])
                lns = []
                for t in range(ST):
                    g = s * ST + t
                    ln_t = lnp.tile([P, H], _BF16, name=f"ln_{g}", tag="ln_t")
                    nc.vector.tensor_scalar(
                        out=ln_t, in0=h_sup[:, t, :],
                        scalar1=mv[:, t, 0:1], scalar2=mv[:, t, 1:2],
                        op0=mybir.AluOpType.subtract, op1=mybir.AluOpType.mult,
                    )
                    lns.append(ln_t)
                h_sups[s] = h_sup
                ln_ts[s] = lns
                lnTs[s] = lntp.tile([P, KO1, ST_TOK], _BF16, name=f"lnT{s}",
                                    tag="lnT")

            def emit_a2_one(s, idx):
                """one PE transpose + DVE psum->sbuf copy (idx in [0, ST*KO1))"""
                t, k = divmod(idx, KO1)
                trp = ps_tr.tile([P, P], _BF16, name=f"tr_{s}_{idx}", tag="trp")
                nc.tensor.transpose(trp, ln_ts[s][t][:, k * P:(k + 1) * P], ident)
                nc.scalar.copy(out=lnTs[s][:, k, t * P:(t + 1) * P], in_=trp)

            def emit_b(s, interleave_a2):
                """GEMM1 + bias + exact GELU -> interT; the next supertile's
                transposes ride along in the last i-chunks so their ACT
                copies precede the tail GELUs in queue order."""
                interT = intp.tile([P, IC, ST_TOK], _BF16, name=f"interT{s}",
                                   tag="interT")
                lnT = lnTs[s]
                a2_idx = 0
                for i in range(IC):
                    pg1 = ps_g1.tile([P, ST_TOK], _F32, name=f"pg1_{s}_{i}",
                                     tag="pg1")
                    for k in range(KO1):
                        nc.tensor.matmul(pg1,
                                         w1_ig[i // ICG][:, k, (i % ICG) * P:
                                                         (i % ICG + 1) * P],
                                         lnT[:, k, :],
                                         start=(k == 0), stop=(k == KO1 - 1))
                    if interleave_a2 is not None and i >= IC - IG:
                        for _ in range(ST * KO1 // IG):
                            if a2_idx < ST * KO1:
                                emit_a2_one(interleave_a2, a2_idx)
                                a2_idx += 1
                    nc.scalar.activation(out=interT[:, i, :], in_=pg1,
                                         func=mybir.ActivationFunctionType.Gelu,
                                         bias=b1_st[:, i:i + 1], scale=1.0)
                return interT

            def emit_c(s, interT):
                """GEMM2 (W2 streamed) + epilogue"""
                for hc in range(HC):
                    pg2s = [ps_g2.tile([P, HCW], _F32, name=f"pg2_{s}_{hc}_{tq}",
                                       tag="pg2")
                            for tq in range(ST)]
                    for i in range(IC):
                        w2c = w2s.tile([P, HCW], _BF16, name=f"w2c_{s}_{hc}_{i}",
                                       tag="w2c")
                        nc.sync.dma_start(out=w2c,
                                          in_=w2r[:, i, hc * HCW:(hc + 1) * HCW])
                        for tq in range(ST):
                            nc.tensor.matmul(pg2s[tq],
                                             interT[:, i, tq * P:(tq + 1) * P],
                                             w2c,
                                             start=(i == 0), stop=(i == IC - 1))
                    for tq in range(ST):
                        g = s * ST + tq
                        res_h = resp.tile([P, HCW], _F32, name=f"res_{s}_{hc}_{tq}",
                                          tag="res_h")
                        nc.vector.tensor_add(res_h, pg2s[tq],
                                             h_sups[s][:, tq, hc * HCW:(hc + 1) * HCW])
                        nc.vector.tensor_add(res_h, res_h,
                                             ob_full[:, hc * HCW:(hc + 1) * HCW])
                        nc.gpsimd.dma_start(
                            out=out[g * P:(g + 1) * P, hc * HCW:(hc + 1) * HCW],
                            in_=res_h)

            # ---- emission schedule ----
            emit_a1(0)                      # token loads queue first on sync
            for ig in range(IG):            # W1 on the gpsimd queue, in 8 groups
                w1t = w1p.tile([P, KO1, ICG * P], _BF16, name=f"w1_{ig}",
                               tag=f"w1_{ig}")
                # two strided 3D DMAs per group: SWDGE issue rate is the
                # scarce resource (~0.65us/instruction), not transfer size
                kh = KO1 // 2
                for q in range(2):
                    nc.gpsimd.dma_start(
                        out=w1t[:, q * kh:(q + 1) * kh, :],
                        in_=w1r[:, q * kh:(q + 1) * kh,
                                ig * ICG * P:(ig + 1) * ICG * P])
                w1_ig[ig] = w1t
            for idx in range(ST * KO1):     # supertile 0 transposes up front
                emit_a2_one(0, idx)
            for s in range(N_SUPER):
                if s + 1 < N_SUPER:
                    emit_a1(s + 1)
                interT = emit_b(s, s + 1 if s + 1 < N_SUPER else None)
                emit_c(s, interT)

    nc.finalize()
    return nc


def kernel(input, residual, bias, attn_nw, attn_nb, inter_w, inter_b,
           output_w, output_b):
    global LAST_RESULT
    input = np.asarray(input, dtype=np.float32)
    residual = np.asarray(residual, dtype=np.float32)
    bias = np.asarray(bias, dtype=np.float32)
    attn_nw = np.asarray(attn_nw, dtype=np.float32)
    attn_nb = np.asarray(attn_nb, dtype=np.float32)
    inter_w = np.asarray(inter_w, dtype=np.float32)
    inter_b = np.asarray(inter_b, dtype=np.float32)
    output_w = np.asarray(output_w, dtype=np.float32)
    output_b = np.asarray(output_b, dtype=np.float32)

    x = np.ascontiguousarray(input.reshape(TOK, H))
    r = np.ascontiguousarray(residual.reshape(TOK, H))
    # fold LN affine params into GEMM1 weight/bias (exact algebra):
    #   (std*nw + nb) @ W1 + b1 == std @ (nw[:,None]*W1) + (nb @ W1 + b1)
    w1p = np.ascontiguousarray((attn_nw[:, None] * inter_w)).astype(ml_dtypes.bfloat16)
    b1p = (attn_nb @ inter_w + inter_b).astype(np.float32)
    w2p = np.ascontiguousarray(output_w).astype(ml_dtypes.bfloat16)
    eye = np.eye(P, dtype=ml_dtypes.bfloat16)

    nc = _build_nc()
    in_maps = []
    for c in range(N_CORES):
        in_maps.append({
            "x": np.ascontiguousarray(x[c * TPC:(c + 1) * TPC]),
            "r": np.ascontiguousarray(r[c * TPC:(c + 1) * TPC]),
            "w1": w1p, "b1": b1p, "w2": w2p,
            "ab": bias, "ob": output_b, "eye": eye,
        })
    res = run_bass_kernel_spmd(nc, in_maps, core_ids=list(range(N_CORES)),
                               trace=TRACE)
    LAST_RESULT = res
    out = np.concatenate([res.results[c]["out"] for c in range(N_CORES)], axis=0)
    return np.ascontiguousarray(out.reshape(B, S, H)).astype(np.float32)
